# revision 25
# baseline (speedup 1.0000x reference)
"""Trainium2 Bass kernel for the LRU (Linear Recurrent Unit) nn.Module.

Math
----
Reference computes, per timestep t (T=4096, H=2048, N=1024):
    Bu_t   = B_norm @ u_t                    (complex, B_norm = (B_re+iB_im)*gamma)
    h_t    = lambda * h_{t-1} + Bu_t         (diagonal complex recurrence)
    y_t    = Re(C @ h_t) + D * u_t

Device strategy (8 NeuronCores, tensor-parallel over d_hidden N):
Each core owns NSH = N/8 = 128 channels.  With lambda_n = r_n * exp(i*theta_n)
the rotating-frame substitution g_t = exp(-i*theta*t) * h_t turns the complex
recurrence into two *real* scans
    g_t = r * g_{t-1} + exp(-i*theta*t) * Bu_t
which map 1:1 onto the VectorE tensor_tensor_scan instruction.  Rotation
tables cos(theta_n*t), sin(theta_n*t) are precomputed on host in float64.

All matmul operands (x, B, C, h) are bf16: fp32r matmuls run as two bf16
passes on the PE, so bf16 halves TensorE time; bf16 x / y / tables also
halve the HBM traffic, which is at the roofline otherwise.  PSUM
accumulation and the scan state stay fp32.

Per core:
  mm1  (TensorE):  Bu.T = BnT.T @ x.T        -> [NSH, T] (re,im) in PSUM
  rot-in (VectorE): w = exp(-i theta t) Bu   -> SBUF (bf16)
  scan (VectorE):  g = scan(r, w)            (chunked, carried via `initial`)
  rot-out (VectorE): h = exp(+i theta t) g   -> bf16 SBUF
  mm2  (TensorE):  y_part = h_re.T @ C_re.T - h_im.T @ C_im.T  -> [T, H]
Host gathers: y = sum_c y_part_c + D * u.

Pipeline: three emission phases per chunk, staggered so every engine queue
(strict FIFO each) sees its work in the order it can actually run:

    for c: front(c) [x DMA + mm1]; vec(c-1) [rot/scan]; back(c-2) [mm2+copy]

TensorE gets mm1(c) two chunks ahead of mm2(c-2); the VectorE queue gets
back(c)'s PSUM evacuation copies *before* vec(c+2)'s rotation chain so mm2
bank reuse is never gated behind a whole chunk of rotation work.  The PSUM
evacuations are split 3:1 scalar:vector so the scalar stream keeps pace
with the mm2 matmuls.
"""

import os

import numpy as np

T, H, N = 4096, 2048, 1024
NCORES = 8
NSH = N // NCORES  # 128 channels per core
TCH = 512          # time chunk (= max fp32 matmul moving free dim = 1 PSUM bank)
NCHUNK = T // TCH  # 8
KT = H // 128      # 16 contraction tiles in mm1
HCH = 512          # h chunk in mm2
NHC = H // HCH     # 4

_CACHE = {}

# last BassKernelResults (for test harness introspection)
last_results = None


def _build_program():
    import concourse.mybir as mybir
    from concourse import bacc
    from concourse.tile import TileContext

    F32 = mybir.dt.float32
    BF16 = mybir.dt.bfloat16
    MUL = mybir.AluOpType.mult
    ADD = mybir.AluOpType.add
    SUB = mybir.AluOpType.subtract

    nc = bacc.Bacc("TRN2", target_bir_lowering=False, debug=False,
                   num_devices=NCORES)

    xT = nc.dram_tensor("xT", [128, NCHUNK * KT * TCH], BF16,
                        kind="ExternalInput").ap()
    bn_re = nc.dram_tensor("bn_re", [128, KT * NSH], BF16,
                           kind="ExternalInput").ap()
    bn_im = nc.dram_tensor("bn_im", [128, KT * NSH], BF16,
                           kind="ExternalInput").ap()
    ct_re = nc.dram_tensor("ct_re", [NSH, H], BF16, kind="ExternalInput").ap()
    ct_in = nc.dram_tensor("ct_in", [NSH, H], BF16, kind="ExternalInput").ap()
    cosT = nc.dram_tensor("cosT", [NSH, T], BF16, kind="ExternalInput").ap()
    sinT = nc.dram_tensor("sinT", [NSH, T], BF16, kind="ExternalInput").ap()
    rvec = nc.dram_tensor("rvec", [NSH, 1], F32, kind="ExternalInput").ap()
    ypart = nc.dram_tensor("ypart", [T, H], BF16, kind="ExternalOutput").ap()

    with TileContext(nc) as tc:
        with (
            tc.tile_pool(name="persist", bufs=1) as pp,
            tc.tile_pool(name="xin", bufs=4) as xp,
            tc.tile_pool(name="rot", bufs=2) as rp,
            tc.tile_pool(name="wbuf", bufs=3) as wp,
            tc.tile_pool(name="gbuf", bufs=3) as gp,
            tc.tile_pool(name="hbuf", bufs=3) as hp,
            tc.tile_pool(name="yout", bufs=3) as yp,
            tc.tile_pool(name="csn", bufs=3) as cp,
            tc.tile_pool(name="ps1", bufs=2, space="PSUM") as ps1,
            tc.tile_pool(name="ps2", bufs=4, space="PSUM") as ps2,
        ):
            # ---- persistent loads ----
            # bn_re/bn_im feed the first matmuls.  Emit them as quarter-DMAs
            # interleaved with chunk 0's x quarters (see emit_front) so the
            # very first matmul only waits on ~0.75 MB, not the full weight
            # set.  rvec (a tiny strided transfer) and the C loads go to the
            # gpsimd queue: they are needed only by the scan / emit_back,
            # two-plus chunks in.
            bre = pp.tile([128, KT * NSH], BF16, tag="bre")
            bim = pp.tile([128, KT * NSH], BF16, tag="bim")
            BQ = KT * NSH // 4

            def load_b_quarter(q):
                nc.sync.dma_start(bre[:, q * BQ:(q + 1) * BQ],
                                  bn_re[:, q * BQ:(q + 1) * BQ])
                nc.sync.dma_start(bim[:, q * BQ:(q + 1) * BQ],
                                  bn_im[:, q * BQ:(q + 1) * BQ])

            rv = pp.tile([128, 1], F32, tag="rv")
            nc.gpsimd.dma_start(rv[:], rvec)
            ctr = pp.tile([128, H], BF16, tag="ctr")
            cti = pp.tile([128, H], BF16, tag="cti")
            rbc = pp.tile([128, TCH], F32, tag="rbc")
            nc.vector.tensor_copy(rbc[:], rv[:, 0:1].broadcast_to([128, TCH]))

            state = {}  # chunk -> (pre, pim, csl, snl); plus scan carry
            hist = {}   # chunk -> (hre, him) awaiting mm2

            def emit_front(c):
                """x DMA + mm1 for chunk c (TensorE + sync DMA queue)."""
                t0 = c * TCH
                xt = xp.tile([128, KT * TCH], BF16, tag="xt", bufs=4)
                x0 = c * KT * TCH
                QW = KT * TCH // 4
                for q in range(4):
                    if c == 0 and q == 0:
                        # first transfer split in two so the very first
                        # matmul's dependency is only ~0.4 MB
                        E = QW // 2
                        nc.sync.dma_start(xt[:, 0:E], xT[:, x0:x0 + E])
                        load_b_quarter(0)
                        nc.sync.dma_start(xt[:, E:QW], xT[:, x0 + E:x0 + QW])
                        continue
                    nc.sync.dma_start(
                        xt[:, q * QW:(q + 1) * QW],
                        xT[:, x0 + q * QW:x0 + (q + 1) * QW],
                    )
                    if c == 0:
                        # weight quarters ride between chunk 0's x quarters
                        load_b_quarter(q)
                # rotation tables prefetch one chunk ahead of the vec phase
                csl_t = cp.tile([128, TCH], BF16, tag="csl")
                snl_t = cp.tile([128, TCH], BF16, tag="snl")
                nc.gpsimd.dma_start(csl_t[:], cosT[:, t0:t0 + TCH])
                nc.gpsimd.dma_start(snl_t[:], sinT[:, t0:t0 + TCH])
                if c == 1:
                    # C loads ride the gpsimd queue behind chunk 1's tables:
                    # late enough to keep the head DMA window (x + B + chunk
                    # 0 tables) uncontended, early enough for emit_back(0)
                    nc.gpsimd.dma_start(ctr[:], ct_re)
                    nc.gpsimd.dma_start(cti[:], ct_in)
                pre = ps1.tile([128, TCH], F32, tag="pre")
                pim = ps1.tile([128, TCH], F32, tag="pim")
                for a in range(KT):
                    xsl = xt[:, a * TCH:(a + 1) * TCH]
                    nc.tensor.matmul(
                        pre[:], bre[:, a * NSH:(a + 1) * NSH], xsl,
                        start=(a == 0), stop=(a == KT - 1),
                    )
                    nc.tensor.matmul(
                        pim[:], bim[:, a * NSH:(a + 1) * NSH], xsl,
                        start=(a == 0), stop=(a == KT - 1),
                    )
                state[c] = (pre, pim, csl_t, snl_t)

            def emit_vec(c):
                """rot-in + scans + rot-out for chunk c (VectorE)."""
                pre, pim, csl_t, snl_t = state.pop(c)
                csl = csl_t[:]
                snl = snl_t[:]
                # rotate into the r-frame: w = e^{-i theta t} * Bu
                t1 = rp.tile([128, TCH], BF16, tag="t1", bufs=2)
                t2 = rp.tile([128, TCH], BF16, tag="t2", bufs=2)
                wre = wp.tile([128, TCH], BF16, tag="wre")
                wim = wp.tile([128, TCH], BF16, tag="wim")
                nc.vector.tensor_tensor(t1[:], csl, pre[:], op=MUL)
                nc.vector.tensor_tensor(t2[:], snl, pim[:], op=MUL)
                nc.vector.tensor_tensor(wre[:], t1[:], t2[:], op=ADD)
                nc.vector.tensor_tensor(t1[:], csl, pim[:], op=MUL)
                nc.vector.tensor_tensor(t2[:], snl, pre[:], op=MUL)
                nc.vector.tensor_tensor(wim[:], t1[:], t2[:], op=SUB)
                # the two real scans (state fp32 internally, bf16 out)
                gre = gp.tile([128, TCH], BF16, tag="gre")
                gim = gp.tile([128, TCH], BF16, tag="gim")
                init_re = 0.0 if c == 0 else state["gre"][:, TCH - 1:TCH]
                init_im = 0.0 if c == 0 else state["gim"][:, TCH - 1:TCH]
                nc.vector.tensor_tensor_scan(
                    gre[:], rbc[:], wre[:], init_re, MUL, ADD)
                nc.vector.tensor_tensor_scan(
                    gim[:], rbc[:], wim[:], init_im, MUL, ADD)
                state["gre"], state["gim"] = gre, gim
                state[("rot", c)] = (csl_t, snl_t, gre, gim)

            def emit_rotout(c, eng):
                """rotate back: h = e^{+i theta t} * g.  All-SBUF bf16 ops,
                normally on GpSimd — VectorE needs the headroom for the
                rot-in (PSUM-bound) + scans + its share of mm2 evacuations.
                The final chunk's rot-out goes on VectorE (emitted after
                back(NCHUNK-2)'s evacuations): GpSimd's ~1.2us/op chain
                would serialize the pipeline drain."""
                csl_t, snl_t, gre, gim = state.pop(("rot", c))
                csl = csl_t[:]
                snl = snl_t[:]
                u1 = rp.tile([128, TCH], BF16, tag="u1", bufs=2)
                u2 = rp.tile([128, TCH], BF16, tag="u2", bufs=2)
                hre = hp.tile([128, TCH], BF16, tag="hre")
                him = hp.tile([128, TCH], BF16, tag="him")
                eng.tensor_tensor(u1[:], csl, gre[:], op=MUL)
                eng.tensor_tensor(u2[:], snl, gim[:], op=MUL)
                eng.tensor_tensor(hre[:], u1[:], u2[:], op=SUB)
                eng.tensor_tensor(u1[:], csl, gim[:], op=MUL)
                eng.tensor_tensor(u2[:], snl, gre[:], op=MUL)
                eng.tensor_tensor(him[:], u1[:], u2[:], op=ADD)
                hist[c] = (hre, him)

            def emit_back(c):
                """mm2 + PSUM evacuation + output DMA for chunk c."""
                hre, him = hist.pop(c)
                t0 = c * TCH
                for tt in range(TCH // 128):
                    lre = hre[:, tt * 128:(tt + 1) * 128]
                    lim = him[:, tt * 128:(tt + 1) * 128]
                    yo = yp.tile([128, H], BF16, tag="yo")
                    for hc in range(NHC):
                        po = ps2.tile([128, HCH], F32, tag="po")
                        nc.tensor.matmul(
                            po[:], lre, ctr[:, hc * HCH:(hc + 1) * HCH],
                            start=True, stop=False,
                        )
                        nc.tensor.matmul(
                            po[:], lim, cti[:, hc * HCH:(hc + 1) * HCH],
                            start=False, stop=True,
                        )
                        # 2+2 scalar/vector keeps both evacuation streams
                        # under the mm2 matmul pace
                        if hc % 2 == 1:
                            nc.vector.tensor_copy(
                                yo[:, hc * HCH:(hc + 1) * HCH], po[:])
                        else:
                            nc.scalar.copy(
                                yo[:, hc * HCH:(hc + 1) * HCH], po[:])
                    nc.sync.dma_start(
                        ypart[t0 + tt * 128:t0 + (tt + 1) * 128, :], yo[:])

            for c in range(NCHUNK):
                emit_front(c)
                if c >= 1:
                    emit_vec(c - 1)
                    emit_rotout(c - 1, nc.gpsimd)
                if c >= 2:
                    emit_back(c - 2)
            emit_vec(NCHUNK - 1)
            emit_back(NCHUNK - 2)
            emit_rotout(NCHUNK - 1, nc.vector)
            emit_back(NCHUNK - 1)

    nc.compile()
    return nc


def _bf16(a):
    import ml_dtypes
    return np.ascontiguousarray(a).astype(ml_dtypes.bfloat16)


def _arrange_bn(bn_slice):
    # bn_slice [NSH, H] (float64) -> [128, KT*NSH] with
    # out[p, a*NSH + n] = bn_slice[n, a*128 + p]
    bnT = bn_slice.T.astype(np.float32)  # [H, NSH]
    return np.ascontiguousarray(
        bnT.reshape(KT, 128, NSH).transpose(1, 0, 2)).reshape(128, -1)


def _host_prep(inputs, nu, theta, gamma_log, B_re, B_im, C_re, C_im, D):
    """Float64 host-side precompute; returns per-core input maps."""
    x = np.asarray(inputs, dtype=np.float32)
    th64 = np.exp(np.asarray(theta).astype(np.float64))
    r64 = np.exp(-np.exp(np.asarray(nu).astype(np.float64)))
    gamma = np.exp(np.asarray(gamma_log).astype(np.float64))
    Bn_re = np.asarray(B_re).astype(np.float64) * gamma[:, None]
    Bn_im = np.asarray(B_im).astype(np.float64) * gamma[:, None]
    t_idx = np.arange(T, dtype=np.float64)
    phase = th64[:, None] * t_idx[None, :]
    cos_all = np.cos(phase).astype(np.float32)  # [N, T]
    sin_all = np.sin(phase).astype(np.float32)
    # pre-arrange x into the per-chunk SBUF layout:
    # xTa[p, c, a, t] = x[c*TCH + t, a*128 + p]
    xTa = _bf16(
        x.reshape(NCHUNK, TCH, KT, 128).transpose(3, 0, 2, 1).reshape(128, -1))
    C_re = np.asarray(C_re, dtype=np.float32)
    C_im = np.asarray(C_im, dtype=np.float32)

    in_maps = []
    for c in range(NCORES):
        sl = slice(c * NSH, (c + 1) * NSH)
        in_maps.append({
            "xT": xTa,
            "bn_re": _bf16(_arrange_bn(Bn_re[sl])),
            "bn_im": _bf16(_arrange_bn(Bn_im[sl])),
            "ct_re": _bf16(C_re[:, sl].T),
            "ct_in": _bf16(-C_im[:, sl].T),
            "cosT": _bf16(cos_all[sl]),
            "sinT": _bf16(sin_all[sl]),
            "rvec": np.ascontiguousarray(r64[sl].astype(np.float32)[:, None]),
        })
    return in_maps


def kernel(inputs, nu, theta, gamma_log, B_re, B_im, C_re, C_im, D):
    global last_results
    from concourse.bass_utils import run_bass_kernel_spmd

    if "nc" not in _CACHE:
        _CACHE["nc"] = _build_program()
    nc = _CACHE["nc"]

    in_maps = _host_prep(
        inputs, nu, theta, gamma_log, B_re, B_im, C_re, C_im, D)

    trace = os.environ.get("LRU_TRACE") == "1"
    res = run_bass_kernel_spmd(
        nc, in_maps, core_ids=list(range(NCORES)), trace=trace)
    last_results = res

    y64 = np.zeros((T, H), np.float64)
    for r in res.results:
        y64 += np.asarray(r["ypart"]).astype(np.float64)
    y64 += (np.asarray(D).astype(np.float64)[None, :]
            * np.asarray(inputs).astype(np.float64))
    return y64.astype(np.float32)


# revision 27
# speedup vs baseline: 1.0201x; 1.0201x over previous
"""Trainium2 Bass kernel for the LRU (Linear Recurrent Unit) nn.Module.

Math
----
Reference computes, per timestep t (T=4096, H=2048, N=1024):
    Bu_t   = B_norm @ u_t                    (complex, B_norm = (B_re+iB_im)*gamma)
    h_t    = lambda * h_{t-1} + Bu_t         (diagonal complex recurrence)
    y_t    = Re(C @ h_t) + D * u_t

Device strategy (8 NeuronCores, tensor-parallel over d_hidden N):
Each core owns NSH = N/8 = 128 channels.  With lambda_n = r_n * exp(i*theta_n)
the rotating-frame substitution g_t = exp(-i*theta*t) * h_t turns the complex
recurrence into two *real* scans
    g_t = r * g_{t-1} + exp(-i*theta*t) * Bu_t
which map 1:1 onto the VectorE tensor_tensor_scan instruction.  Rotation
tables cos(theta_n*t), sin(theta_n*t) are precomputed on host in float64.

All matmul operands (x, B, C, h) are bf16: fp32r matmuls run as two bf16
passes on the PE, so bf16 halves TensorE time; bf16 x / y / tables also
halve the HBM traffic, which is at the roofline otherwise.  PSUM
accumulation and the scan state stay fp32.

Per core:
  mm1  (TensorE):  Bu.T = BnT.T @ x.T        -> [NSH, T] (re,im) in PSUM
  rot-in (VectorE): w = exp(-i theta t) Bu   -> SBUF (bf16)
  scan (VectorE):  g = scan(r, w)            (chunked, carried via `initial`)
  rot-out (VectorE): h = exp(+i theta t) g   -> bf16 SBUF
  mm2  (TensorE):  y_part = h_re.T @ C_re.T - h_im.T @ C_im.T  -> [T, H]
Host gathers: y = sum_c y_part_c + D * u.

Pipeline: three emission phases per chunk, staggered so every engine queue
(strict FIFO each) sees its work in the order it can actually run:

    for c: front(c) [x DMA + mm1]; vec(c-1) [rot/scan]; back(c-2) [mm2+copy]

TensorE gets mm1(c) two chunks ahead of mm2(c-2); the VectorE queue gets
back(c)'s PSUM evacuation copies *before* vec(c+2)'s rotation chain so mm2
bank reuse is never gated behind a whole chunk of rotation work.  The PSUM
evacuations are split 3:1 scalar:vector so the scalar stream keeps pace
with the mm2 matmuls.
"""

import os

import numpy as np

T, H, N = 4096, 2048, 1024
NCORES = 8
NSH = N // NCORES  # 128 channels per core
TCH = 512          # time chunk (= max fp32 matmul moving free dim = 1 PSUM bank)
NCHUNK = T // TCH  # 8
KT = H // 128      # 16 contraction tiles in mm1
HCH = 512          # h chunk in mm2
NHC = H // HCH     # 4

_CACHE = {}

# last BassKernelResults (for test harness introspection)
last_results = None


def _build_program():
    import concourse.mybir as mybir
    from concourse import bacc
    from concourse.tile import TileContext

    F32 = mybir.dt.float32
    BF16 = mybir.dt.bfloat16
    MUL = mybir.AluOpType.mult
    ADD = mybir.AluOpType.add
    SUB = mybir.AluOpType.subtract

    nc = bacc.Bacc("TRN2", target_bir_lowering=False, debug=False,
                   num_devices=NCORES)

    xT = nc.dram_tensor("xT", [128, NCHUNK * KT * TCH], BF16,
                        kind="ExternalInput").ap()
    bn_re = nc.dram_tensor("bn_re", [128, KT * NSH], BF16,
                           kind="ExternalInput").ap()
    bn_im = nc.dram_tensor("bn_im", [128, KT * NSH], BF16,
                           kind="ExternalInput").ap()
    ct_re = nc.dram_tensor("ct_re", [NSH, H], BF16, kind="ExternalInput").ap()
    ct_in = nc.dram_tensor("ct_in", [NSH, H], BF16, kind="ExternalInput").ap()
    cosT = nc.dram_tensor("cosT", [NSH, T], BF16, kind="ExternalInput").ap()
    sinT = nc.dram_tensor("sinT", [NSH, T], BF16, kind="ExternalInput").ap()
    rvec = nc.dram_tensor("rvec", [NSH, 1], F32, kind="ExternalInput").ap()
    ypart = nc.dram_tensor("ypart", [T, H], BF16, kind="ExternalOutput").ap()

    with TileContext(nc) as tc:
        with (
            tc.tile_pool(name="persist", bufs=1) as pp,
            tc.tile_pool(name="xin", bufs=4) as xp,
            tc.tile_pool(name="rot", bufs=2) as rp,
            tc.tile_pool(name="wbuf", bufs=3) as wp,
            tc.tile_pool(name="gbuf", bufs=3) as gp,
            tc.tile_pool(name="hbuf", bufs=3) as hp,
            tc.tile_pool(name="yout", bufs=3) as yp,
            tc.tile_pool(name="csn", bufs=3) as cp,
            tc.tile_pool(name="ps1", bufs=2, space="PSUM") as ps1,
            tc.tile_pool(name="ps2", bufs=4, space="PSUM") as ps2,
        ):
            # ---- persistent loads ----
            # bn_re/bn_im feed the first matmuls.  Emit them as quarter-DMAs
            # interleaved with chunk 0's x quarters (see emit_front) so the
            # very first matmul only waits on ~0.75 MB, not the full weight
            # set.  rvec (a tiny strided transfer) and the C loads go to the
            # gpsimd queue: they are needed only by the scan / emit_back,
            # two-plus chunks in.
            bre = pp.tile([128, KT * NSH], BF16, tag="bre")
            bim = pp.tile([128, KT * NSH], BF16, tag="bim")
            BQ = KT * NSH // 4

            def load_b_quarter(q):
                nc.sync.dma_start(bre[:, q * BQ:(q + 1) * BQ],
                                  bn_re[:, q * BQ:(q + 1) * BQ])
                nc.sync.dma_start(bim[:, q * BQ:(q + 1) * BQ],
                                  bn_im[:, q * BQ:(q + 1) * BQ])

            rv = pp.tile([128, 1], F32, tag="rv")
            nc.gpsimd.dma_start(rv[:], rvec)
            ctr = pp.tile([128, H], BF16, tag="ctr")
            cti = pp.tile([128, H], BF16, tag="cti")
            rbc = pp.tile([128, TCH], F32, tag="rbc")
            nc.vector.tensor_copy(rbc[:], rv[:, 0:1].broadcast_to([128, TCH]))

            state = {}  # chunk -> (pre, pim, csl, snl); plus scan carry
            hist = {}   # chunk -> (hre, him) awaiting mm2

            def emit_front(c):
                """x DMA + mm1 for chunk c (TensorE + sync DMA queue)."""
                t0 = c * TCH
                xt = xp.tile([128, KT * TCH], BF16, tag="xt", bufs=4)
                x0 = c * KT * TCH
                QW = KT * TCH // 4
                for q in range(4):
                    if c == 0 and q == 0:
                        # first transfer split in two so the very first
                        # matmul's dependency is only ~0.4 MB
                        E = QW // 2
                        nc.sync.dma_start(xt[:, 0:E], xT[:, x0:x0 + E])
                        load_b_quarter(0)
                        nc.sync.dma_start(xt[:, E:QW], xT[:, x0 + E:x0 + QW])
                        continue
                    nc.sync.dma_start(
                        xt[:, q * QW:(q + 1) * QW],
                        xT[:, x0 + q * QW:x0 + (q + 1) * QW],
                    )
                    if c == 0:
                        # weight quarters ride between chunk 0's x quarters
                        load_b_quarter(q)
                # rotation tables prefetch one chunk ahead of the vec phase
                csl_t = cp.tile([128, TCH], BF16, tag="csl")
                snl_t = cp.tile([128, TCH], BF16, tag="snl")
                nc.gpsimd.dma_start(csl_t[:], cosT[:, t0:t0 + TCH])
                nc.gpsimd.dma_start(snl_t[:], sinT[:, t0:t0 + TCH])
                if c == 0:
                    # C loads ride the gpsimd queue behind chunk 0's tables;
                    # they are first needed by emit_back(0), two chunks later
                    nc.gpsimd.dma_start(ctr[:], ct_re)
                    nc.gpsimd.dma_start(cti[:], ct_in)
                pre = ps1.tile([128, TCH], F32, tag="pre")
                pim = ps1.tile([128, TCH], F32, tag="pim")
                for a in range(KT):
                    xsl = xt[:, a * TCH:(a + 1) * TCH]
                    nc.tensor.matmul(
                        pre[:], bre[:, a * NSH:(a + 1) * NSH], xsl,
                        start=(a == 0), stop=(a == KT - 1),
                    )
                    nc.tensor.matmul(
                        pim[:], bim[:, a * NSH:(a + 1) * NSH], xsl,
                        start=(a == 0), stop=(a == KT - 1),
                    )
                state[c] = (pre, pim, csl_t, snl_t)

            def emit_vec(c):
                """rot-in + scans + rot-out for chunk c (VectorE)."""
                pre, pim, csl_t, snl_t = state.pop(c)
                csl = csl_t[:]
                snl = snl_t[:]
                # rotate into the r-frame: w = e^{-i theta t} * Bu
                t1 = rp.tile([128, TCH], BF16, tag="t1", bufs=2)
                t2 = rp.tile([128, TCH], BF16, tag="t2", bufs=2)
                wre = wp.tile([128, TCH], BF16, tag="wre")
                wim = wp.tile([128, TCH], BF16, tag="wim")
                nc.vector.tensor_tensor(t1[:], csl, pre[:], op=MUL)
                nc.vector.tensor_tensor(t2[:], snl, pim[:], op=MUL)
                nc.vector.tensor_tensor(wre[:], t1[:], t2[:], op=ADD)
                nc.vector.tensor_tensor(t1[:], csl, pim[:], op=MUL)
                nc.vector.tensor_tensor(t2[:], snl, pre[:], op=MUL)
                nc.vector.tensor_tensor(wim[:], t1[:], t2[:], op=SUB)
                # the two real scans (state fp32 internally, bf16 out)
                gre = gp.tile([128, TCH], BF16, tag="gre")
                gim = gp.tile([128, TCH], BF16, tag="gim")
                init_re = 0.0 if c == 0 else state["gre"][:, TCH - 1:TCH]
                init_im = 0.0 if c == 0 else state["gim"][:, TCH - 1:TCH]
                nc.vector.tensor_tensor_scan(
                    gre[:], rbc[:], wre[:], init_re, MUL, ADD)
                nc.vector.tensor_tensor_scan(
                    gim[:], rbc[:], wim[:], init_im, MUL, ADD)
                state["gre"], state["gim"] = gre, gim
                state[("rot", c)] = (csl_t, snl_t, gre, gim)

            def emit_rotout(c, eng):
                """rotate back: h = e^{+i theta t} * g.  All-SBUF bf16 ops,
                normally on GpSimd — VectorE needs the headroom for the
                rot-in (PSUM-bound) + scans + its share of mm2 evacuations.
                The final chunk's rot-out goes on VectorE (emitted after
                back(NCHUNK-2)'s evacuations): GpSimd's ~1.2us/op chain
                would serialize the pipeline drain."""
                csl_t, snl_t, gre, gim = state.pop(("rot", c))
                csl = csl_t[:]
                snl = snl_t[:]
                u1 = rp.tile([128, TCH], BF16, tag="u1", bufs=2)
                u2 = rp.tile([128, TCH], BF16, tag="u2", bufs=2)
                hre = hp.tile([128, TCH], BF16, tag="hre")
                him = hp.tile([128, TCH], BF16, tag="him")
                eng.tensor_tensor(u1[:], csl, gre[:], op=MUL)
                eng.tensor_tensor(u2[:], snl, gim[:], op=MUL)
                eng.tensor_tensor(hre[:], u1[:], u2[:], op=SUB)
                eng.tensor_tensor(u1[:], csl, gim[:], op=MUL)
                eng.tensor_tensor(u2[:], snl, gre[:], op=MUL)
                eng.tensor_tensor(him[:], u1[:], u2[:], op=ADD)
                hist[c] = (hre, him)

            def emit_back(c):
                """mm2 + PSUM evacuation + output DMA for chunk c."""
                hre, him = hist.pop(c)
                t0 = c * TCH
                for tt in range(TCH // 128):
                    lre = hre[:, tt * 128:(tt + 1) * 128]
                    lim = him[:, tt * 128:(tt + 1) * 128]
                    yo = yp.tile([128, H], BF16, tag="yo")
                    for half in range(2):
                        po2 = ps2.tile([128, 2 * HCH], F32, tag="po", bufs=2)
                        for k in range(2):
                            hc = 2 * half + k
                            sl = po2[:, k * HCH:(k + 1) * HCH]
                            nc.tensor.matmul(
                                sl, lre, ctr[:, hc * HCH:(hc + 1) * HCH],
                                start=True, stop=False,
                            )
                            nc.tensor.matmul(
                                sl, lim, cti[:, hc * HCH:(hc + 1) * HCH],
                                start=False, stop=True,
                            )
                        if half == 1:
                            nc.vector.tensor_copy(
                                yo[:, 2 * half * HCH:(2 * half + 2) * HCH],
                                po2[:])
                        else:
                            nc.scalar.copy(
                                yo[:, 2 * half * HCH:(2 * half + 2) * HCH],
                                po2[:])
                    nc.sync.dma_start(
                        ypart[t0 + tt * 128:t0 + (tt + 1) * 128, :], yo[:])

            for c in range(NCHUNK):
                emit_front(c)
                if c >= 1:
                    emit_vec(c - 1)
                    emit_rotout(c - 1, nc.gpsimd)
                if c >= 2:
                    emit_back(c - 2)
            emit_vec(NCHUNK - 1)
            emit_back(NCHUNK - 2)
            emit_rotout(NCHUNK - 1, nc.vector)
            emit_back(NCHUNK - 1)

    nc.compile()
    return nc


def _bf16(a):
    import ml_dtypes
    return np.ascontiguousarray(a).astype(ml_dtypes.bfloat16)


def _arrange_bn(bn_slice):
    # bn_slice [NSH, H] (float64) -> [128, KT*NSH] with
    # out[p, a*NSH + n] = bn_slice[n, a*128 + p]
    bnT = bn_slice.T.astype(np.float32)  # [H, NSH]
    return np.ascontiguousarray(
        bnT.reshape(KT, 128, NSH).transpose(1, 0, 2)).reshape(128, -1)


def _host_prep(inputs, nu, theta, gamma_log, B_re, B_im, C_re, C_im, D):
    """Float64 host-side precompute; returns per-core input maps."""
    x = np.asarray(inputs, dtype=np.float32)
    th64 = np.exp(np.asarray(theta).astype(np.float64))
    r64 = np.exp(-np.exp(np.asarray(nu).astype(np.float64)))
    gamma = np.exp(np.asarray(gamma_log).astype(np.float64))
    Bn_re = np.asarray(B_re).astype(np.float64) * gamma[:, None]
    Bn_im = np.asarray(B_im).astype(np.float64) * gamma[:, None]
    t_idx = np.arange(T, dtype=np.float64)
    phase = th64[:, None] * t_idx[None, :]
    cos_all = np.cos(phase).astype(np.float32)  # [N, T]
    sin_all = np.sin(phase).astype(np.float32)
    # pre-arrange x into the per-chunk SBUF layout:
    # xTa[p, c, a, t] = x[c*TCH + t, a*128 + p]
    xTa = _bf16(
        x.reshape(NCHUNK, TCH, KT, 128).transpose(3, 0, 2, 1).reshape(128, -1))
    C_re = np.asarray(C_re, dtype=np.float32)
    C_im = np.asarray(C_im, dtype=np.float32)

    in_maps = []
    for c in range(NCORES):
        sl = slice(c * NSH, (c + 1) * NSH)
        in_maps.append({
            "xT": xTa,
            "bn_re": _bf16(_arrange_bn(Bn_re[sl])),
            "bn_im": _bf16(_arrange_bn(Bn_im[sl])),
            "ct_re": _bf16(C_re[:, sl].T),
            "ct_in": _bf16(-C_im[:, sl].T),
            "cosT": _bf16(cos_all[sl]),
            "sinT": _bf16(sin_all[sl]),
            "rvec": np.ascontiguousarray(r64[sl].astype(np.float32)[:, None]),
        })
    return in_maps


def kernel(inputs, nu, theta, gamma_log, B_re, B_im, C_re, C_im, D):
    global last_results
    from concourse.bass_utils import run_bass_kernel_spmd

    if "nc" not in _CACHE:
        _CACHE["nc"] = _build_program()
    nc = _CACHE["nc"]

    in_maps = _host_prep(
        inputs, nu, theta, gamma_log, B_re, B_im, C_re, C_im, D)

    trace = os.environ.get("LRU_TRACE") == "1"
    res = run_bass_kernel_spmd(
        nc, in_maps, core_ids=list(range(NCORES)), trace=trace)
    last_results = res

    y64 = np.zeros((T, H), np.float64)
    for r in res.results:
        y64 += np.asarray(r["ypart"]).astype(np.float64)
    y64 += (np.asarray(D).astype(np.float64)[None, :]
            * np.asarray(inputs).astype(np.float64))
    return y64.astype(np.float32)
